# revision 12
# baseline (speedup 1.0000x reference)
"""Routed low-rank FFN (MoE-style) Trainium2 kernel.

out[n] = x[n] @ U[pids[n]] @ V[pids[n]] + bias

Strategy (expert-parallel over 8 NeuronCores), expert-PAIRED groups:
  - Host: tokens grouped by pid; expert p's tokens go to core p // 8.
    Each core's 8 experts are paired (largest with smallest) into G=4
    groups.  Group capacities (c1, c) are shared across cores (SPMD
    program identical everywhere); expert 1 occupies token columns
    [0, c1), expert 2 [c1, c).
  - mm1 (h = x @ U): one [128, 128] fp8 weight tile per k-chunk holds
    BOTH experts' U columns; streaming the pair's tokens once computes
    both experts' h rows; the wrong-expert half of each PSUM column is
    garbage that hcast zeroes.
  - hcast: PSUM -> fp16 hT tile [128, c]: rows 0:64 = expert-1 h (cols
    [0,c1)), rows 64:128 = expert-2 h (cols [c1,c)); wrong-expert
    blocks memset to 0 so mm2's contraction kills the garbage.
  - mm2 (out^T = V.T @ h^T): weights = [V_p; V_q] d_out-slice
    [128, 128] fp8, rhs = hT streams the pair's tokens; 8 slices ->
    out^T in PSUM as 8 x [128, c].  One stream serves both experts:
    half the stream cycles of per-expert mm2.
  - Epilogue: PSUM -> fp8 packed [128, 8*c] split across Scalar/
    Vector/GpSimd; one contiguous store per pair on the scalar ring
    (sync ring is loads-only, so neither ring's FIFO mixes directions);
    final pair split into two half stores on separate rings.
  - PE queue order puts mm2(g-2) (data already on-chip) BEFORE mm1(g)
    (may still be in flight) to avoid head-of-line DMA stalls, and a
    short warmup burst keeps the HAM clock-gate fed during the DMA
    head.
  - fp8 e4m3 inputs (x, U, V), fp16 h and out, f32 PSUM.  Bias is added
    entirely on the host (f32, exact), so no ones-row matmul.
  - Host: inverse-permute rows, add bias, cast f32.
"""

import os

import numpy as np
import ml_dtypes

N_CORES = 8
D_IN = 1024
RANK = 64
D_OUT = 1024
KC = 8  # number of 128-deep contraction chunks: D_IN // 128
NSL = 8  # number of 128-wide d_out slices: D_OUT // 128

F8 = ml_dtypes.float8_e4m3
F16 = np.float16

# Set by kernel() after a traced run (KERNEL_TRACE=1): HW kernel span in ns.
LAST_EXEC_TIME_NS = None
LAST_RESULTS = None

_PROGRAM_CACHE = {}


def _layout(G, cvec):
    """Per-pair input layout: [x-chunks (KC*c) | U-chunks (KC*128) | V (1024)]."""
    fws = [KC * c + KC * 128 + D_OUT for (_, c) in cvec]
    offs = [sum(fws[:g]) for g in range(G + 1)]
    return fws, offs


def _build_program(G: int, cvec: tuple):
    """SPMD Bass/Tile program: G expert-pair groups per core; group g has
    capacities cvec[g] = (c1, c)."""
    import concourse.tile as tile
    from concourse import bacc, mybir

    nc = bacc.Bacc(
        "TRN2",
        target_bir_lowering=False,
        debug=False,
        enable_asserts=False,
        num_devices=N_CORES,
    )
    f32 = mybir.dt.float32
    f16 = mybir.dt.float16
    f8 = mybir.dt.float8e4

    C0 = max(c for (_, c) in cvec)
    fws, offs = _layout(G, cvec)
    IW = offs[G]
    ows = [NSL * c for (_, c) in cvec]
    ooffs = [sum(ows[:g]) for g in range(G + 1)]
    OW = ooffs[G]

    in_d = nc.dram_tensor("ing", [128, IW], f8, kind="ExternalInput")
    u8 = mybir.dt.uint8
    o_d = nc.dram_tensor("og", [128, OW], u8, kind="ExternalOutput")

    # load slices (column bounds): pair0 x+U first (smallest latency to
    # first matmul), then pair0 V + pair1, then pair2, pair3.
    sb = [0, KC * cvec[0][1] + KC * 128, offs[2], offs[4]]

    with tile.TileContext(nc) as tc:
        with (
            tc.tile_pool(name="xin", bufs=1) as xpool,
            tc.tile_pool(name="win", bufs=1) as wpool,
            tc.tile_pool(name="hbuf", bufs=1) as hpool,
            tc.tile_pool(name="obuf", bufs=2) as opool,
            tc.tile_pool(name="ph", bufs=1, space="PSUM") as phpool,
            tc.tile_pool(name="po", bufs=2, space="PSUM") as popool,
            tc.tile_pool(name="wm", bufs=1, space="PSUM") as wmpool,
        ):
            # All loads on the sync ring, first thing, in consumption order.
            parts = []
            for s in range(len(sb) - 1):
                t = xpool.tile([128, sb[s + 1] - sb[s]], f8, tag=f"in{s}")
                nc.sync.dma_start(out=t[:], in_=in_d[:, sb[s] : sb[s + 1]])
                parts.append(t)

            def region(col):
                s = next(i for i in range(len(sb) - 1) if sb[i + 1] > col)
                return parts[s], col - sb[s]

            # HAM warmup: a short dense burst bridging the DMA head.
            wm_lhs = wpool.tile([128, 32], f16, tag="wml")
            wm_rhs = wpool.tile([128, 512], f16, tag="wmr")
            nc.gpsimd.memset(wm_lhs[:], 0.0)
            nc.gpsimd.memset(wm_rhs[:], 0.0)
            wm_ps = wmpool.tile([32, 512], f32, tag="wm")
            for _ in range(6):
                nc.tensor.matmul(
                    wm_ps[:], lhsT=wm_lhs[:], rhs=wm_rhs[:], start=True, stop=True
                )

            def keep_warm():
                nc.tensor.matmul(
                    wm_ps[:, 0:256],
                    lhsT=wm_lhs[:],
                    rhs=wm_rhs[:, 0:256],
                    start=True,
                    stop=True,
                )

            hTs = [
                hpool.tile([128, C0], f16, tag=f"h{i}", name=f"hT{i}")
                for i in range(3)
            ]

            phs, pos, o_sbs = [None] * G, [None] * G, [None] * G

            def mm1(g):
                c1, cg = cvec[g]
                xt, x0 = region(offs[g])
                ut, u0 = region(offs[g] + KC * cg)
                ph = phpool.tile([128, C0], f32, tag=f"ph{g % 2}", name=f"ph{g}")
                phs[g] = ph
                for k in range(KC):
                    nc.tensor.matmul(
                        ph[:, :cg],
                        lhsT=ut[:, u0 + k * 128 : u0 + (k + 1) * 128],
                        rhs=xt[:, x0 + k * cg : x0 + (k + 1) * cg],
                        start=(k == 0),
                        stop=(k == KC - 1),
                    )

            def hcast(g):
                # fp16 round + wrong-expert zeroing; copies on VectorE,
                # zero blocks on GpSimd.
                c1, cg = cvec[g]
                hT = hTs[g % 3]
                ph = phs[g]
                nc.vector.tensor_copy(hT[0:RANK, 0:c1], ph[0:RANK, 0:c1])
                nc.vector.tensor_copy(hT[RANK:128, c1:cg], ph[RANK:128, c1:cg])
                nc.gpsimd.memset(hT[RANK:128, 0:c1], 0.0)
                nc.gpsimd.memset(hT[0:RANK, c1:cg], 0.0)

            def mm2(g):
                # 8 d_out slices; slices (2j, 2j+1) share a single-bank
                # PSUM tile at col offsets 0 / C0.
                c1, cg = cvec[g]
                vt, v0 = region(offs[g] + KC * cg + KC * 128)
                pot = [
                    popool.tile(
                        [128, 2 * C0], f32, tag=f"po{j % 2}", name=f"po{j}_{g}"
                    )
                    for j in range(NSL // 2)
                ]
                pos[g] = pot
                hT = hTs[g % 3]
                for j in range(NSL):
                    nc.tensor.matmul(
                        pot[j // 2][:, (j % 2) * C0 : (j % 2) * C0 + cg],
                        lhsT=vt[:, v0 + j * 128 : v0 + (j + 1) * 128],
                        rhs=hT[:, :cg],
                        start=True,
                        stop=True,
                    )

            def epilogue_store(g, last):
                # PSUM -> packed fp16 [128, NSL*cg]; slices split across
                # Scalar (0-2), Vector (3-5), GpSimd (6-7); one store per
                # group on the scalar ring (final group split over both).
                c1, cg = cvec[g]
                pot = pos[g]
                o_sb = opool.tile([128, NSL * C0], f8, tag="o", name=f"o{g}")
                o_sbs[g] = o_sb
                osb3 = o_sb[:, : NSL * cg].rearrange(
                    "p (j c) -> p j c", j=NSL, c=cg
                )
                for t in range(NSL // 2):
                    # one strided copy moves both slices of po tile t
                    src = pot[t][:].rearrange("p (s c) -> p s c", s=2, c=C0)[
                        :, :, :cg
                    ]
                    dst = osb3[:, 2 * t : 2 * t + 2, :]
                    if t < 2:
                        nc.scalar.copy(dst, src)
                    else:
                        nc.vector.tensor_copy(dst, src)
                go = ooffs[g]
                if last:
                    half = NSL // 2 * cg
                    nc.scalar.dma_start(
                        out=o_d[:, go : go + half],
                        in_=o_sb[:, 0:half].bitcast(u8),
                    )
                    nc.sync.dma_start(
                        out=o_d[:, go + half : go + NSL * cg],
                        in_=o_sb[:, half : NSL * cg].bitcast(u8),
                    )
                else:
                    eng = nc.scalar if g % 2 == 0 else nc.sync
                    eng.dma_start(
                        out=o_d[:, go : go + NSL * cg],
                        in_=o_sb[:, : NSL * cg].bitcast(u8),
                    )

            # depth-2 software pipeline; mm2(g-2) (data on-chip) is queued
            # BEFORE mm1(g) (DMA may still be in flight) so the in-order PE
            # queue never head-of-line blocks on a load.
            depth = min(2, G - 1) if G > 1 else 0
            for g in range(depth):
                mm1(g)
                hcast(g)
            for g in range(depth, G):
                mm2(g - depth)
                if g < G - 1:
                    keep_warm()
                mm1(g)
                hcast(g)
                epilogue_store(g - depth, last=False)
            for g in range(G - depth, G):
                mm2(g)
                epilogue_store(g, last=(g == G - 1))

    nc.compile()
    return nc


def _route(pids: np.ndarray, n_experts: int):
    """Group token indices by expert; pair experts within each core
    (largest with smallest) into G = experts_per_core // 2 groups."""
    order = np.argsort(pids, kind="stable")
    counts = np.bincount(pids, minlength=n_experts)
    offs = np.concatenate([[0], np.cumsum(counts)])
    per_core = n_experts // N_CORES
    core_groups = []
    for c in range(N_CORES):
        experts = list(range(c * per_core, (c + 1) * per_core))
        experts.sort(key=lambda p: -counts[p])
        pairs = []
        for i in range(per_core // 2):
            p, q = experts[i], experts[per_core - 1 - i]
            pairs.append(
                (p, order[offs[p] : offs[p + 1]], q, order[offs[q] : offs[q + 1]])
            )
        pairs.sort(key=lambda t: -(len(t[1]) + len(t[3])))
        core_groups.append(pairs)
    return core_groups


def _capacity(core_groups):
    """G and shared per-group (c1, c) capacities."""
    G = max(len(gs) for gs in core_groups)

    def r8(v):
        return max(8, -(-v // 8) * 8)

    cvec = []
    for g in range(G):
        c1 = r8(max(len(gs[g][1]) for gs in core_groups))
        c2 = r8(max(len(gs[g][3]) for gs in core_groups))
        cvec.append((c1, c1 + c2))
    return G, tuple(cvec)


def _pack_core(groups, G, cvec, x8, U8, V8):
    """Build one core's in_map from its (p, toks_p, q, toks_q) groups."""
    fws, offs = _layout(G, cvec)
    ing = np.zeros((128, offs[G]), F8)
    for g, (p, tp, q, tq) in enumerate(groups):
        c1, cg = cvec[g]
        w0 = offs[g]
        blk = np.zeros((cg, D_IN), F8)
        blk[: len(tp)] = x8[tp]
        blk[c1 : c1 + len(tq)] = x8[tq]
        # [c, d] -> [d, c] -> [k, dp, c] -> [dp, k, c] -> [128, KC*c]
        ing[:, w0 : w0 + KC * cg] = (
            blk.T.reshape(KC, 128, cg).transpose(1, 0, 2).reshape(128, KC * cg)
        )
        U2 = np.concatenate([U8[p], U8[q]], axis=1)  # [1024, 128]
        ing[:, w0 + KC * cg : w0 + KC * cg + KC * 128] = (
            U2.reshape(KC, 128, 128).transpose(1, 0, 2).reshape(128, KC * 128)
        )
        ing[:, w0 + KC * cg + KC * 128 : offs[g + 1]] = np.concatenate(
            [V8[p], V8[q]], axis=0
        )
    return {"ing": ing}


def _unpack(og_list, core_groups, cvec, N, bias):
    """og [128, OW] per core -> out rows + bias (full f32 bias add)."""
    G = len(cvec)
    ows = [NSL * c for (_, c) in cvec]
    ooffs = [sum(ows[:g]) for g in range(G + 1)]
    out = np.empty((N, D_OUT), np.float32)
    for c in range(N_CORES):
        og = np.asarray(og_list[c])
        if og.dtype == np.uint8:
            og = og.view(F8)
        og = og.astype(np.float32)
        for g, (p, tp, q, tq) in enumerate(core_groups[c]):
            c1, cg = cvec[g]
            sub = og[:, ooffs[g] : ooffs[g] + NSL * cg]
            # [o, j, c] -> [c, j, o] -> [c, 1024]
            res = (
                sub.reshape(128, NSL, cg).transpose(2, 1, 0).reshape(cg, D_OUT)
            )
            out[tp] = res[: len(tp)]
            out[tq] = res[c1 : c1 + len(tq)]
    return out + bias


def kernel(x, pids, U, V, bias):
    global LAST_EXEC_TIME_NS, LAST_RESULTS
    from concourse.bass_utils import run_bass_kernel_spmd

    x = np.asarray(x, dtype=np.float32)
    pids_np = np.asarray(pids).astype(np.int64)
    U = np.asarray(U, dtype=np.float32)
    V = np.asarray(V, dtype=np.float32)
    bias = np.asarray(bias, dtype=np.float32)

    N = x.shape[0]
    P = U.shape[0]

    x8 = x.astype(F8)
    U8 = U.astype(F8)
    V8 = V.astype(F8)

    core_groups = _route(pids_np, P)
    G, cvec = _capacity(core_groups)

    in_maps = [
        _pack_core(core_groups[c], G, cvec, x8, U8, V8) for c in range(N_CORES)
    ]

    key = (G, cvec)
    if key not in _PROGRAM_CACHE:
        _PROGRAM_CACHE[key] = _build_program(G, cvec)
    nc = _PROGRAM_CACHE[key]

    trace = os.environ.get("KERNEL_TRACE", "0") == "1"
    res = run_bass_kernel_spmd(nc, in_maps, list(range(N_CORES)), trace=trace)
    LAST_EXEC_TIME_NS = res.exec_time_ns
    LAST_RESULTS = res

    return _unpack(
        [res.results[c]["og"] for c in range(N_CORES)], core_groups, cvec, N, bias
    )


# revision 13
# speedup vs baseline: 1.0623x; 1.0623x over previous
"""Routed low-rank FFN (MoE-style) Trainium2 kernel.

out[n] = x[n] @ U[pids[n]] @ V[pids[n]] + bias

Strategy (expert-parallel over 8 NeuronCores), expert-PAIRED groups:
  - Host: tokens grouped by pid; expert p's tokens go to core p // 8.
    Each core's 8 experts are paired (largest with smallest) into G=4
    groups.  Group capacities (c1, c) are shared across cores (SPMD
    program identical everywhere); expert 1 occupies token columns
    [0, c1), expert 2 [c1, c).
  - mm1 (h = x @ U): one [128, 128] fp8 weight tile per k-chunk holds
    BOTH experts' U columns; streaming the pair's tokens once computes
    both experts' h rows; the wrong-expert half of each PSUM column is
    garbage that hcast zeroes.
  - hcast: PSUM -> fp16 hT tile [128, c]: rows 0:64 = expert-1 h (cols
    [0,c1)), rows 64:128 = expert-2 h (cols [c1,c)); wrong-expert
    blocks memset to 0 so mm2's contraction kills the garbage.
  - mm2 (out^T = V.T @ h^T): weights = [V_p; V_q] d_out-slice
    [128, 128] fp8, rhs = hT streams the pair's tokens; 8 slices ->
    out^T in PSUM as 8 x [128, c].  One stream serves both experts:
    half the stream cycles of per-expert mm2.
  - Epilogue: PSUM -> fp8 packed [128, 8*c] split across Scalar/
    Vector/GpSimd; one contiguous store per pair on the scalar ring
    (sync ring is loads-only, so neither ring's FIFO mixes directions);
    final pair split into two half stores on separate rings.
  - PE queue order puts mm2(g-2) (data already on-chip) BEFORE mm1(g)
    (may still be in flight) to avoid head-of-line DMA stalls, and a
    short warmup burst keeps the HAM clock-gate fed during the DMA
    head.
  - fp8 e4m3 inputs (x, U, V), fp16 h and out, f32 PSUM.  Bias is added
    entirely on the host (f32, exact), so no ones-row matmul.
  - Host: inverse-permute rows, add bias, cast f32.
"""

import os

import numpy as np
import ml_dtypes

N_CORES = 8
D_IN = 1024
RANK = 64
D_OUT = 1024
KC = 8  # number of 128-deep contraction chunks: D_IN // 128
NSL = 8  # number of 128-wide d_out slices: D_OUT // 128

F8 = ml_dtypes.float8_e4m3
F16 = np.float16

# Set by kernel() after a traced run (KERNEL_TRACE=1): HW kernel span in ns.
LAST_EXEC_TIME_NS = None
LAST_RESULTS = None

_PROGRAM_CACHE = {}


def _layout(G, cvec):
    """Per-pair input layout: [x-chunks (KC*c) | U-chunks (KC*128) | V (1024)]."""
    fws = [KC * c + KC * 128 + D_OUT for (_, c) in cvec]
    offs = [sum(fws[:g]) for g in range(G + 1)]
    return fws, offs


def _build_program(G: int, cvec: tuple):
    """SPMD Bass/Tile program: G expert-pair groups per core; group g has
    capacities cvec[g] = (c1, c)."""
    import concourse.tile as tile
    from concourse import bacc, mybir

    nc = bacc.Bacc(
        "TRN2",
        target_bir_lowering=False,
        debug=False,
        enable_asserts=False,
        num_devices=N_CORES,
    )
    f32 = mybir.dt.float32
    f16 = mybir.dt.float16
    f8 = mybir.dt.float8e4

    C0 = max(c for (_, c) in cvec)
    fws, offs = _layout(G, cvec)
    IW = offs[G]
    ows = [NSL * c for (_, c) in cvec]
    ooffs = [sum(ows[:g]) for g in range(G + 1)]
    OW = ooffs[G]

    in_d = nc.dram_tensor("ing", [128, IW], f8, kind="ExternalInput")
    u8 = mybir.dt.uint8
    o_d = nc.dram_tensor("og", [128, OW], u8, kind="ExternalOutput")

    # load slices (column bounds): pair0 x+U first (smallest latency to
    # first matmul), then pair0 V + pair1, then pair2, pair3.
    sb = [0, offs[1], offs[2], offs[4]]

    with tile.TileContext(nc) as tc:
        with (
            tc.tile_pool(name="xin", bufs=1) as xpool,
            tc.tile_pool(name="win", bufs=1) as wpool,
            tc.tile_pool(name="hbuf", bufs=1) as hpool,
            tc.tile_pool(name="obuf", bufs=2) as opool,
            tc.tile_pool(name="ph", bufs=1, space="PSUM") as phpool,
            tc.tile_pool(name="po", bufs=2, space="PSUM") as popool,
            tc.tile_pool(name="wm", bufs=1, space="PSUM") as wmpool,
        ):
            # All loads on the sync ring, first thing, in consumption order.
            parts = []
            for s in range(len(sb) - 1):
                t = xpool.tile([128, sb[s + 1] - sb[s]], f8, tag=f"in{s}")
                nc.sync.dma_start(out=t[:], in_=in_d[:, sb[s] : sb[s + 1]])
                parts.append(t)

            def region(col):
                s = next(i for i in range(len(sb) - 1) if sb[i + 1] > col)
                return parts[s], col - sb[s]

            # HAM warmup: a short dense burst bridging the DMA head.
            wm_lhs = wpool.tile([128, 32], f16, tag="wml")
            wm_rhs = wpool.tile([128, 512], f16, tag="wmr")
            nc.gpsimd.memset(wm_lhs[:], 0.0)
            nc.gpsimd.memset(wm_rhs[:], 0.0)
            wm_ps = wmpool.tile([32, 512], f32, tag="wm")
            for _ in range(8):
                nc.tensor.matmul(
                    wm_ps[:], lhsT=wm_lhs[:], rhs=wm_rhs[:], start=True, stop=True
                )

            def keep_warm():
                nc.tensor.matmul(
                    wm_ps[:, 0:256],
                    lhsT=wm_lhs[:],
                    rhs=wm_rhs[:, 0:256],
                    start=True,
                    stop=True,
                )

            hTs = [
                hpool.tile([128, C0], f16, tag=f"h{i}", name=f"hT{i}")
                for i in range(3)
            ]

            phs, pos, o_sbs = [None] * G, [None] * G, [None] * G

            def mm1(g):
                c1, cg = cvec[g]
                xt, x0 = region(offs[g])
                ut, u0 = region(offs[g] + KC * cg)
                ph = phpool.tile([128, C0], f32, tag=f"ph{g % 2}", name=f"ph{g}")
                phs[g] = ph
                for k in range(KC):
                    nc.tensor.matmul(
                        ph[:, :cg],
                        lhsT=ut[:, u0 + k * 128 : u0 + (k + 1) * 128],
                        rhs=xt[:, x0 + k * cg : x0 + (k + 1) * cg],
                        start=(k == 0),
                        stop=(k == KC - 1),
                    )

            def hcast(g):
                # fp16 round + wrong-expert zeroing; copies on VectorE,
                # zero blocks on GpSimd.
                c1, cg = cvec[g]
                hT = hTs[g % 3]
                ph = phs[g]
                nc.vector.tensor_copy(hT[0:RANK, 0:c1], ph[0:RANK, 0:c1])
                nc.vector.tensor_copy(hT[RANK:128, c1:cg], ph[RANK:128, c1:cg])
                nc.gpsimd.memset(hT[RANK:128, 0:c1], 0.0)
                nc.gpsimd.memset(hT[0:RANK, c1:cg], 0.0)

            def mm2(g):
                # 8 d_out slices; slices (2j, 2j+1) share a single-bank
                # PSUM tile at col offsets 0 / C0.
                c1, cg = cvec[g]
                vt, v0 = region(offs[g] + KC * cg + KC * 128)
                pot = [
                    popool.tile(
                        [128, 2 * C0], f32, tag=f"po{j % 2}", name=f"po{j}_{g}"
                    )
                    for j in range(NSL // 2)
                ]
                pos[g] = pot
                hT = hTs[g % 3]
                for j in range(NSL):
                    nc.tensor.matmul(
                        pot[j // 2][:, (j % 2) * C0 : (j % 2) * C0 + cg],
                        lhsT=vt[:, v0 + j * 128 : v0 + (j + 1) * 128],
                        rhs=hT[:, :cg],
                        start=True,
                        stop=True,
                    )

            def epilogue_store(g, last):
                # PSUM -> packed fp16 [128, NSL*cg]; slices split across
                # Scalar (0-2), Vector (3-5), GpSimd (6-7); one store per
                # group on the scalar ring (final group split over both).
                c1, cg = cvec[g]
                pot = pos[g]
                o_sb = opool.tile([128, NSL * C0], f8, tag="o", name=f"o{g}")
                o_sbs[g] = o_sb
                osb3 = o_sb[:, : NSL * cg].rearrange(
                    "p (j c) -> p j c", j=NSL, c=cg
                )
                for t in range(NSL // 2):
                    # one strided copy moves both slices of po tile t
                    src = pot[t][:].rearrange("p (s c) -> p s c", s=2, c=C0)[
                        :, :, :cg
                    ]
                    dst = osb3[:, 2 * t : 2 * t + 2, :]
                    if t < 2:
                        nc.scalar.copy(dst, src)
                    else:
                        nc.vector.tensor_copy(dst, src)
                go = ooffs[g]
                if last:
                    half = NSL // 2 * cg
                    nc.scalar.dma_start(
                        out=o_d[:, go : go + half],
                        in_=o_sb[:, 0:half].bitcast(u8),
                    )
                    nc.sync.dma_start(
                        out=o_d[:, go + half : go + NSL * cg],
                        in_=o_sb[:, half : NSL * cg].bitcast(u8),
                    )
                else:
                    eng = nc.scalar if g % 2 == 0 else nc.sync
                    eng.dma_start(
                        out=o_d[:, go : go + NSL * cg],
                        in_=o_sb[:, : NSL * cg].bitcast(u8),
                    )

            # depth-2 software pipeline; mm2(g-2) (data on-chip) is queued
            # BEFORE mm1(g) (DMA may still be in flight) so the in-order PE
            # queue never head-of-line blocks on a load.
            depth = min(2, G - 1) if G > 1 else 0
            for g in range(depth):
                mm1(g)
                hcast(g)
                for _ in range(2 - g):
                    keep_warm()
            for g in range(depth, G):
                mm2(g - depth)
                if g < G - 1:
                    keep_warm()
                mm1(g)
                hcast(g)
                epilogue_store(g - depth, last=False)
            for g in range(G - depth, G):
                mm2(g)
                epilogue_store(g, last=(g == G - 1))

    nc.compile()
    return nc


def _route(pids: np.ndarray, n_experts: int):
    """Group token indices by expert; pair experts within each core
    (largest with smallest) into G = experts_per_core // 2 groups."""
    order = np.argsort(pids, kind="stable")
    counts = np.bincount(pids, minlength=n_experts)
    offs = np.concatenate([[0], np.cumsum(counts)])
    per_core = n_experts // N_CORES
    core_groups = []
    for c in range(N_CORES):
        experts = list(range(c * per_core, (c + 1) * per_core))
        experts.sort(key=lambda p: -counts[p])
        pairs = []
        for i in range(per_core // 2):
            p, q = experts[i], experts[per_core - 1 - i]
            pairs.append(
                (p, order[offs[p] : offs[p + 1]], q, order[offs[q] : offs[q + 1]])
            )
        pairs.sort(key=lambda t: -(len(t[1]) + len(t[3])))
        core_groups.append(pairs)
    return core_groups


def _capacity(core_groups):
    """G and shared per-group (c1, c) capacities."""
    G = max(len(gs) for gs in core_groups)

    def r8(v):
        return max(8, -(-v // 8) * 8)

    cvec = []
    for g in range(G):
        c1 = r8(max(len(gs[g][1]) for gs in core_groups))
        c2 = r8(max(len(gs[g][3]) for gs in core_groups))
        cvec.append((c1, c1 + c2))
    return G, tuple(cvec)


def _pack_core(groups, G, cvec, x8, U8, V8):
    """Build one core's in_map from its (p, toks_p, q, toks_q) groups."""
    fws, offs = _layout(G, cvec)
    ing = np.zeros((128, offs[G]), F8)
    for g, (p, tp, q, tq) in enumerate(groups):
        c1, cg = cvec[g]
        w0 = offs[g]
        blk = np.zeros((cg, D_IN), F8)
        blk[: len(tp)] = x8[tp]
        blk[c1 : c1 + len(tq)] = x8[tq]
        # [c, d] -> [d, c] -> [k, dp, c] -> [dp, k, c] -> [128, KC*c]
        ing[:, w0 : w0 + KC * cg] = (
            blk.T.reshape(KC, 128, cg).transpose(1, 0, 2).reshape(128, KC * cg)
        )
        U2 = np.concatenate([U8[p], U8[q]], axis=1)  # [1024, 128]
        ing[:, w0 + KC * cg : w0 + KC * cg + KC * 128] = (
            U2.reshape(KC, 128, 128).transpose(1, 0, 2).reshape(128, KC * 128)
        )
        ing[:, w0 + KC * cg + KC * 128 : offs[g + 1]] = np.concatenate(
            [V8[p], V8[q]], axis=0
        )
    return {"ing": ing}


def _unpack(og_list, core_groups, cvec, N, bias):
    """og [128, OW] per core -> out rows + bias (full f32 bias add)."""
    G = len(cvec)
    ows = [NSL * c for (_, c) in cvec]
    ooffs = [sum(ows[:g]) for g in range(G + 1)]
    out = np.empty((N, D_OUT), np.float32)
    for c in range(N_CORES):
        og = np.asarray(og_list[c])
        if og.dtype == np.uint8:
            og = og.view(F8)
        og = og.astype(np.float32)
        for g, (p, tp, q, tq) in enumerate(core_groups[c]):
            c1, cg = cvec[g]
            sub = og[:, ooffs[g] : ooffs[g] + NSL * cg]
            # [o, j, c] -> [c, j, o] -> [c, 1024]
            res = (
                sub.reshape(128, NSL, cg).transpose(2, 1, 0).reshape(cg, D_OUT)
            )
            out[tp] = res[: len(tp)]
            out[tq] = res[c1 : c1 + len(tq)]
    return out + bias


def kernel(x, pids, U, V, bias):
    global LAST_EXEC_TIME_NS, LAST_RESULTS
    from concourse.bass_utils import run_bass_kernel_spmd

    x = np.asarray(x, dtype=np.float32)
    pids_np = np.asarray(pids).astype(np.int64)
    U = np.asarray(U, dtype=np.float32)
    V = np.asarray(V, dtype=np.float32)
    bias = np.asarray(bias, dtype=np.float32)

    N = x.shape[0]
    P = U.shape[0]

    x8 = x.astype(F8)
    U8 = U.astype(F8)
    V8 = V.astype(F8)

    core_groups = _route(pids_np, P)
    G, cvec = _capacity(core_groups)

    in_maps = [
        _pack_core(core_groups[c], G, cvec, x8, U8, V8) for c in range(N_CORES)
    ]

    key = (G, cvec)
    if key not in _PROGRAM_CACHE:
        _PROGRAM_CACHE[key] = _build_program(G, cvec)
    nc = _PROGRAM_CACHE[key]

    trace = os.environ.get("KERNEL_TRACE", "0") == "1"
    res = run_bass_kernel_spmd(nc, in_maps, list(range(N_CORES)), trace=trace)
    LAST_EXEC_TIME_NS = res.exec_time_ns
    LAST_RESULTS = res

    return _unpack(
        [res.results[c]["og"] for c in range(N_CORES)], core_groups, cvec, N, bias
    )
